# revision 20
# baseline (speedup 1.0000x reference)
"""Bahdanau attention Trainium2 Bass kernel.

Problem (hardcoded): encoder_outputs (32, 4096, 512) f32, decoder_hidden
(32, 512) f32, plus tiny linear params.  Returns (context (32, 512),
attention_weights (32, 4096)).

Sharding: data-parallel over batch across 8 NeuronCores (4 batches/core),
params replicated.  The host pre-casts E to bf16 and feeds it TRANSPOSED
(b, hidden, seq) so no on-chip transposes are needed; HBM traffic per core
is ~17MB (read once).

Per-core dataflow (4 local batches x 8 seq-blocks of 512):
  - enc_proj X^T = W_enc-chunks @ E^T via bf16 matmuls accumulated in PSUM
    (h_out on partitions, seq on free), ScalarE evacuates PSUM with tanh,
    fusing the per-(h,batch) bias (dec_proj + b_enc) as the per-partition
    activation bias; output tt is bf16.
  - score = v^T @ tanh via M=1 bf16 matmuls (b_v dropped: softmax is
    shift-invariant).
  - Online softmax-free context: scores are bounded (|score| <=
    sum|W_v| ~ 23) so exp needs no max-subtraction; per block ScalarE
    computes unnormalized w = exp(score) (+ running sum via accum_out),
    GpSimd broadcasts w across partitions, and VectorE accumulates
    ctx_u[h] += sum_s w(s) E^T(h,s) via scalar_tensor_tensor accum -
    the E^T tile is consumed inside the block pipeline, so batches
    overlap with no end-of-batch stall.
  - Tail per batch: l = sum of block sums, attn = w/l (DMA'd out),
    ctx = ctx_u/l via a reciprocal broadcast + one tiny PE transpose to
    restore natural h-order.

Engine balance (measured): PE ~163us (the bottleneck: 512 enc matmuls
N=512 + 128 score matmuls), ACT ~115us, DVE ~99us, GpSimd ~50us, DMA
~55us/queue; HW exec ~197-201us on 8 cores.
"""
import sys

if "/opt/trn_rl_repo" not in sys.path:
    sys.path.insert(0, "/opt/trn_rl_repo")

from contextlib import ExitStack

import numpy as np

import concourse.bass as bass
import concourse.bacc as bacc
import concourse.tile as tile
import concourse.mybir as mybir
from concourse import bass_utils
from concourse.masks import make_identity

F32 = mybir.dt.float32
BF16 = mybir.dt.bfloat16
AFT = mybir.ActivationFunctionType

HIDDEN = 512
SEQ = 4096
BATCH = 32
N_CORES = 8
B_LOC = BATCH // N_CORES  # 4 batches per core
P = 128
KC = HIDDEN // P  # 4 hidden chunks
SB = 512  # seq block
NBLK = SEQ // SB  # 8 blocks per batch
NT = SEQ // P  # 32 seq tiles per batch


def build_kernel(b_loc=B_LOC, seq=SEQ, hidden=HIDDEN):
    """Build and compile the per-core Bass program."""
    kc = hidden // P
    nblk = seq // SB if seq >= SB else 1
    sb = min(SB, seq)
    nt = seq // P
    tpb = sb // P  # seq tiles per block

    nc = bacc.Bacc("TRN2", target_bir_lowering=False, debug=False)

    enc_t = nc.dram_tensor("enc_t_in", (b_loc, hidden, seq), BF16, kind="ExternalInput").ap()
    dec = nc.dram_tensor("dec_in", (b_loc, hidden), F32, kind="ExternalInput").ap()
    w_enc = nc.dram_tensor("w_enc", (hidden, hidden), BF16, kind="ExternalInput").ap()
    b_enc = nc.dram_tensor("b_enc", (hidden,), F32, kind="ExternalInput").ap()
    w_dec = nc.dram_tensor("w_dec", (hidden, hidden), F32, kind="ExternalInput").ap()
    b_dec = nc.dram_tensor("b_dec", (hidden,), F32, kind="ExternalInput").ap()
    w_v = nc.dram_tensor("w_v", (hidden, 1), BF16, kind="ExternalInput").ap()
    # b_v shifts every score equally; softmax is shift-invariant so it is
    # declared (harness passes it) but unused.
    nc.dram_tensor("b_v", (1,), F32, kind="ExternalInput").ap()

    ctx_out = nc.dram_tensor("ctx_out", (b_loc, hidden), F32, kind="ExternalOutput").ap()
    attn_out = nc.dram_tensor("attn_out", (b_loc, seq), F32, kind="ExternalOutput").ap()

    with tile.TileContext(nc) as tc, ExitStack() as ctx:
        const_pool = ctx.enter_context(tc.tile_pool(name="const", bufs=1))
        # E^T tiles: one per 512-seq block, whole batch resident for the
        # DVE context reduction; +1 so the next batch's first block streams in.
        et_pool = ctx.enter_context(tc.tile_pool(name="et", bufs=nblk + 1))
        tt_pool = ctx.enter_context(tc.tile_pool(name="tt", bufs=3))
        sm_pool = ctx.enter_context(tc.tile_pool(name="smax", bufs=2))

        ps_tr = ctx.enter_context(tc.tile_pool(name="ps_tr", bufs=1, space="PSUM"))
        ps_x = ctx.enter_context(tc.tile_pool(name="ps_x", bufs=4, space="PSUM"))
        ps_sc = ctx.enter_context(tc.tile_pool(name="ps_sc", bufs=2, space="PSUM"))

        # ---- constants / weights ----
        ident = const_pool.tile([P, P], F32)
        make_identity(nc, ident[:])

        w_enc_sb = const_pool.tile([P, kc, hidden], BF16)
        for c in range(kc):
            nc.scalar.dma_start(w_enc_sb[:, c, :], w_enc[c * P : (c + 1) * P, :])
        w_dec_sb = const_pool.tile([P, kc, hidden], F32)
        nc.gpsimd.dma_start(w_dec_sb[:], w_dec.rearrange("(c k) h -> k c h", k=P))
        # biases / v: load as contiguous rows (single-descriptor DMAs), sum,
        # then PE-transpose into per-partition layout
        berow = const_pool.tile([1, hidden], F32)
        nc.gpsimd.dma_start(berow[:], b_enc.rearrange("(o h) -> o h", o=1))
        bdrow = const_pool.tile([1, hidden], F32)
        nc.gpsimd.dma_start(bdrow[:], b_dec.rearrange("(o h) -> o h", o=1))
        bsrow = const_pool.tile([1, hidden], F32)
        nc.vector.tensor_add(bsrow[:], berow[:], bdrow[:])
        vrow = const_pool.tile([1, hidden], BF16)
        nc.gpsimd.dma_start(vrow[:], w_v.rearrange("h o -> o h"))
        ident_bf = const_pool.tile([P, P], BF16)
        nc.vector.tensor_copy(ident_bf[:], ident[:])
        bsum_sb = const_pool.tile([P, kc], F32)
        v_sb = const_pool.tile([P, kc], BF16)
        for c in range(kc):
            psb = ps_tr.tile([P, P], F32, tag="tr")
            nc.tensor.transpose(psb[:, 0:1], bsrow[0:1, c * P : (c + 1) * P], ident[0:1, 0:1])
            nc.vector.tensor_copy(bsum_sb[:, c : c + 1], psb[:, 0:1])
            psv = ps_tr.tile([P, P], BF16, tag="tr")
            nc.tensor.transpose(psv[:, 0:1], vrow[0:1, c * P : (c + 1) * P], ident_bf[0:1, 0:1])
            nc.vector.tensor_copy(v_sb[:, c : c + 1], psv[:, 0:1])

        # ---- decoder projection -> per-(h_chunk, batch) activation bias ----
        # Emitted (via emit_dec_setup) after the first block's enc matmuls so
        # the in-order PE queue reaches the critical-path work first; the
        # first tanh is emitted after the call, so deps stay program-ordered.
        bias_holder = {}

        def emit_dec_setup():
            d_nat = const_pool.tile([b_loc, hidden], F32, name="d_nat")
            nc.gpsimd.dma_start(d_nat[:], dec[:, :])
            dT = const_pool.tile([P, kc, b_loc], F32, name="dT")
            for c in range(kc):
                ps = ps_tr.tile([P, P], F32, tag="tr", name="ps_dec")
                nc.tensor.transpose(ps[:, :b_loc], d_nat[:, c * P : (c + 1) * P], ident[:b_loc, :b_loc])
                nc.vector.tensor_copy(dT[:, c, :], ps[:, :b_loc])
            bias_sb = const_pool.tile([P, kc, b_loc], F32, name="bias_sb")
            bias_holder["bias"] = bias_sb
            for hc in range(kc):
                pd = ps_tr.tile([P, P], F32, tag="tr", name="pd_dec")
                for c in range(kc):
                    nc.tensor.matmul(
                        pd[:, :b_loc],
                        w_dec_sb[:, c, hc * P : (hc + 1) * P],
                        dT[:, c, :],
                        start=(c == 0),
                        stop=(c == kc - 1),
                    )
                nc.vector.tensor_scalar_add(bias_sb[:, hc, :], pd[:, :b_loc], bsum_sb[:, hc : hc + 1])

        # ---- main loop over local batches ----
        for b in range(b_loc):
            wexp = sm_pool.tile([1, seq], F32, tag="wexp")
            lpart = sm_pool.tile([1, nblk], F32, tag="lpart")
            part = sm_pool.tile([P, kc, nblk], F32, tag="part")
            for j in range(nblk):
                # E^T (h_in on partitions) comes pre-transposed from the host
                et = et_pool.tile([P, kc, sb], BF16, tag="et")
                if b == 0 and j < 2:
                    # cold start: spread the first tiles across queues
                    for c in range(kc):
                        nc.sync.dma_start(
                            et[:, c, :],
                            enc_t[b, c * P : (c + 1) * P, j * sb : (j + 1) * sb],
                        )
                else:
                    nc.sync.dma_start(
                        et[:],
                        enc_t[b, :, j * sb : (j + 1) * sb].rearrange("(c k) s -> k c s", k=P),
                    )
                # enc_proj + fused bias+tanh
                tt = tt_pool.tile([P, kc, sb], BF16, tag="tt")
                pxs = []
                for hc in range(kc):
                    px = ps_x.tile([P, SB], F32, tag="x")
                    pxs.append(px)
                    for c in range(kc):
                        nc.tensor.matmul(
                            px[:, :sb],
                            w_enc_sb[:, c, hc * P : (hc + 1) * P],
                            et[:, c, :],
                            start=(c == 0),
                            stop=(c == kc - 1),
                        )
                    if not (b == 0 and j == 0):
                        nc.scalar.activation(
                            tt[:, hc, :], px[:, :sb], AFT.Tanh,
                            bias=bias_holder["bias"][:, hc, b : b + 1],
                        )
                if b == 0 and j == 0:
                    emit_dec_setup()
                    for hc in range(kc):
                        nc.scalar.activation(
                            tt[:, hc, :], pxs[hc][:, :sb], AFT.Tanh,
                            bias=bias_holder["bias"][:, hc, b : b + 1],
                        )
                # score chunk (1, sb)
                pss = ps_sc.tile([1, SB], F32, tag="sc")
                for c in range(kc):
                    nc.tensor.matmul(
                        pss[:, :sb],
                        v_sb[:, c : c + 1],
                        tt[:, c, :],
                        start=(c == 0),
                        stop=(c == kc - 1),
                    )
                # unnormalized softmax weights for this block (scores are
                # bounded, so no max-subtraction is needed) + running sum
                nc.scalar.activation(
                    wexp[:, j * sb : (j + 1) * sb],
                    pss[:, :sb],
                    AFT.Exp,
                    accum_out=lpart[:, j : j + 1],
                )
                # online context accumulation: part[:,c,j] = sum_s w(s)*E^T(h,s)
                bc = sm_pool.tile([P, SB], F32, tag="bc")
                nc.gpsimd.partition_broadcast(bc[:, :sb], wexp[:, j * sb : (j + 1) * sb])
                for c in range(kc):
                    scratch = sm_pool.tile([P, SB], BF16, tag="scr")
                    nc.vector.scalar_tensor_tensor(
                        out=scratch[:, :sb],
                        in0=et[:, c, :],
                        scalar=1.0,
                        in1=bc[:, :sb],
                        op0=mybir.AluOpType.mult,
                        op1=mybir.AluOpType.mult,
                        accum_out=part[:, c, j : j + 1],
                    )

            # ---- per-batch tail: normalize ----
            lsum = sm_pool.tile([1, 1], F32, tag="lsum")
            nc.vector.reduce_sum(lsum[:], lpart[:], axis=mybir.AxisListType.X)
            rl = sm_pool.tile([1, 1], F32, tag="rl")
            nc.vector.reciprocal(rl[:], lsum[:])
            attn_f = sm_pool.tile([1, seq], F32, tag="attn_f")
            nc.vector.tensor_scalar_mul(attn_f[:], wexp[:], rl[:])
            nc.sync.dma_start(attn_out[b : b + 1, :], attn_f[:])

            rlb = sm_pool.tile([P, 1], F32, tag="rlb")
            nc.gpsimd.partition_broadcast(rlb[:], rl[:])
            ctxu = sm_pool.tile([P, kc], F32, tag="ctxu")
            nc.vector.reduce_sum(ctxu[:], part[:], axis=mybir.AxisListType.X)
            ctxT = sm_pool.tile([P, kc], F32, tag="ctxT")
            nc.vector.tensor_scalar_mul(ctxT[:], ctxu[:], rlb[:])
            # ctxT[p, c] = ctx[c*128+p]; one tiny PE transpose to natural order
            pst = ps_tr.tile([P, P], F32, tag="tr")
            nc.tensor.transpose(pst[:kc, :], ctxT[:], ident[:])
            ctx_sb = sm_pool.tile([kc, P], F32, tag="ctx_sb")
            nc.vector.tensor_copy(ctx_sb[:], pst[:kc, :])
            nc.sync.dma_start(ctx_out[b].rearrange("(c k) -> c k", k=P), ctx_sb[:])

    nc.compile()
    return nc


_CACHED_NC = None


def _tf32_round(x: np.ndarray) -> np.ndarray:
    """Round-to-nearest-even to the fp32r (11-bit mantissa) grid; the PE's
    fp32r mode requires pre-rounded operands (bit-matches
    neuron_dtypes.static_cast_fp32_to_fp32r)."""
    b = np.ascontiguousarray(x).view(np.uint32).astype(np.uint64)
    b = (b + 0x7FF + ((b >> 12) & 1)) & np.uint64(0xFFFFF000)
    return b.astype(np.uint32).view(np.float32)



def build_in_maps(inputs) -> list:
    import ml_dtypes

    f = lambda k: np.ascontiguousarray(np.asarray(inputs[k], dtype=np.float32))
    bf = lambda k: np.ascontiguousarray(
        np.asarray(inputs[k], dtype=np.float32).astype(ml_dtypes.bfloat16)
    )
    enc_f32 = np.asarray(inputs["encoder_outputs"], dtype=np.float32)
    enc_t = np.ascontiguousarray(np.swapaxes(enc_f32.astype(ml_dtypes.bfloat16), 1, 2))
    dec = f("decoder_hidden")
    shared = {
        "w_enc": bf("W_enc"),
        "b_enc": f("b_enc"),
        "w_dec": f("W_dec"),
        "b_dec": f("b_dec"),
        "w_v": bf("W_v"),
        "b_v": f("b_v"),
    }
    in_maps = []
    for i in range(N_CORES):
        lo, hi = i * B_LOC, (i + 1) * B_LOC
        in_maps.append(
            {
                "enc_t_in": np.ascontiguousarray(enc_t[lo:hi]),
                "dec_in": np.ascontiguousarray(dec[lo:hi]),
                **shared,
            }
        )
    return in_maps


def kernel(**inputs) -> tuple:
    global _CACHED_NC
    if _CACHED_NC is None:
        _CACHED_NC = build_kernel()
    nc = _CACHED_NC

    in_maps = build_in_maps(inputs)
    res = bass_utils.run_bass_kernel_spmd(nc, in_maps, core_ids=list(range(N_CORES)))
    ctx = np.concatenate([r["ctx_out"] for r in res.results], axis=0)
    attn = np.concatenate([r["attn_out"] for r in res.results], axis=0)
    return ctx, attn


# revision 21
# speedup vs baseline: 1.2234x; 1.2234x over previous
"""Bahdanau attention Trainium2 Bass kernel.

Problem (hardcoded): encoder_outputs (32, 4096, 512) f32, decoder_hidden
(32, 512) f32, plus tiny linear params.  Returns (context (32, 512),
attention_weights (32, 4096)).

Sharding: data-parallel over batch across 8 NeuronCores (4 batches/core),
params replicated.  The host pre-casts E to bf16 and feeds it TRANSPOSED
(b, hidden, seq) so no on-chip transposes are needed; HBM traffic per core
is ~17MB (read once).

Per-core dataflow (4 local batches x 8 seq-blocks of 512):
  - enc_proj X^T = W_enc-chunks @ E^T via bf16 matmuls accumulated in PSUM
    (h_out on partitions, seq on free), ScalarE evacuates PSUM with tanh,
    fusing the per-(h,batch) bias (dec_proj + b_enc) as the per-partition
    activation bias; output tt is bf16.
  - score = v^T @ tanh via M=1 bf16 matmuls (b_v dropped: softmax is
    shift-invariant).
  - Online softmax-free context: scores are bounded (|score| <=
    sum|W_v| ~ 23) so exp needs no max-subtraction; per block ScalarE
    computes unnormalized w = exp(score) (+ running sum via accum_out),
    GpSimd broadcasts w across partitions, and VectorE accumulates
    ctx_u[h] += sum_s w(s) E^T(h,s) via scalar_tensor_tensor accum -
    the E^T tile is consumed inside the block pipeline, so batches
    overlap with no end-of-batch stall.
  - Tail per batch: l = sum of block sums, attn = w/l (DMA'd out),
    ctx = ctx_u/l via a reciprocal broadcast + one tiny PE transpose to
    restore natural h-order.

Engine balance (measured): PE ~163us (the bottleneck: 512 enc matmuls
N=512 + 128 score matmuls), ACT ~115us, DVE ~99us, GpSimd ~50us, DMA
~55us/queue; HW exec ~197-201us on 8 cores.
"""
import sys

if "/opt/trn_rl_repo" not in sys.path:
    sys.path.insert(0, "/opt/trn_rl_repo")

from contextlib import ExitStack

import numpy as np

import concourse.bass as bass
import concourse.bacc as bacc
import concourse.tile as tile
import concourse.mybir as mybir
from concourse import bass_utils
from concourse.masks import make_identity

F32 = mybir.dt.float32
BF16 = mybir.dt.bfloat16
AFT = mybir.ActivationFunctionType

HIDDEN = 512
SEQ = 4096
BATCH = 32
N_CORES = 8
B_LOC = BATCH // N_CORES  # 4 batches per core
P = 128
KC = HIDDEN // P  # 4 hidden chunks
SB = 512  # seq block
NBLK = SEQ // SB  # 8 blocks per batch
NT = SEQ // P  # 32 seq tiles per batch


def build_kernel(b_loc=B_LOC, seq=SEQ, hidden=HIDDEN):
    """Build and compile the per-core Bass program."""
    kc = hidden // P
    nblk = seq // SB if seq >= SB else 1
    sb = min(SB, seq)
    nt = seq // P
    tpb = sb // P  # seq tiles per block

    nc = bacc.Bacc("TRN2", target_bir_lowering=False, debug=False)

    enc_t = nc.dram_tensor("enc_t_in", (b_loc, hidden, seq), BF16, kind="ExternalInput").ap()
    dec = nc.dram_tensor("dec_in", (b_loc, hidden), F32, kind="ExternalInput").ap()
    w_enc = nc.dram_tensor("w_enc", (hidden, hidden), BF16, kind="ExternalInput").ap()
    b_enc = nc.dram_tensor("b_enc", (hidden,), F32, kind="ExternalInput").ap()
    w_dec = nc.dram_tensor("w_dec", (hidden, hidden), F32, kind="ExternalInput").ap()
    b_dec = nc.dram_tensor("b_dec", (hidden,), F32, kind="ExternalInput").ap()
    w_v = nc.dram_tensor("w_v", (hidden, 1), BF16, kind="ExternalInput").ap()
    # b_v shifts every score equally; softmax is shift-invariant so it is
    # declared (harness passes it) but unused.
    nc.dram_tensor("b_v", (1,), F32, kind="ExternalInput").ap()

    ctx_out = nc.dram_tensor("ctx_out", (b_loc, hidden), F32, kind="ExternalOutput").ap()
    attn_out = nc.dram_tensor("attn_out", (b_loc, seq), F32, kind="ExternalOutput").ap()

    with tile.TileContext(nc) as tc, ExitStack() as ctx:
        const_pool = ctx.enter_context(tc.tile_pool(name="const", bufs=1))
        # E^T tiles: one per 512-seq block, whole batch resident for the
        # DVE context reduction; +1 so the next batch's first block streams in.
        et_pool = ctx.enter_context(tc.tile_pool(name="et", bufs=nblk + 1))
        tt_pool = ctx.enter_context(tc.tile_pool(name="tt", bufs=3))
        sm_pool = ctx.enter_context(tc.tile_pool(name="smax", bufs=2))

        ps_tr = ctx.enter_context(tc.tile_pool(name="ps_tr", bufs=1, space="PSUM"))
        ps_x = ctx.enter_context(tc.tile_pool(name="ps_x", bufs=4, space="PSUM"))
        ps_sc = ctx.enter_context(tc.tile_pool(name="ps_sc", bufs=2, space="PSUM"))

        # ---- constants / weights ----
        ident = const_pool.tile([P, P], F32)
        make_identity(nc, ident[:])

        w_enc_sb = const_pool.tile([P, kc, hidden], BF16)
        for c in range(kc):
            nc.scalar.dma_start(w_enc_sb[:, c, :], w_enc[c * P : (c + 1) * P, :])
        w_dec_sb = const_pool.tile([P, kc, hidden], F32)
        nc.gpsimd.dma_start(w_dec_sb[:], w_dec.rearrange("(c k) h -> k c h", k=P))
        # biases / v: load as contiguous rows (single-descriptor DMAs), sum,
        # then PE-transpose into per-partition layout
        berow = const_pool.tile([1, hidden], F32)
        nc.gpsimd.dma_start(berow[:], b_enc.rearrange("(o h) -> o h", o=1))
        bdrow = const_pool.tile([1, hidden], F32)
        nc.gpsimd.dma_start(bdrow[:], b_dec.rearrange("(o h) -> o h", o=1))
        bsrow = const_pool.tile([1, hidden], F32)
        nc.vector.tensor_add(bsrow[:], berow[:], bdrow[:])
        vrow = const_pool.tile([1, hidden], BF16)
        nc.gpsimd.dma_start(vrow[:], w_v.rearrange("h o -> o h"))
        ident_bf = const_pool.tile([P, P], BF16)
        nc.vector.tensor_copy(ident_bf[:], ident[:])
        bsum_sb = const_pool.tile([P, kc], F32)
        v_sb = const_pool.tile([P, kc], BF16)
        for c in range(kc):
            psb = ps_tr.tile([P, P], F32, tag="tr")
            nc.tensor.transpose(psb[:, 0:1], bsrow[0:1, c * P : (c + 1) * P], ident[0:1, 0:1])
            nc.vector.tensor_copy(bsum_sb[:, c : c + 1], psb[:, 0:1])
            psv = ps_tr.tile([P, P], BF16, tag="tr")
            nc.tensor.transpose(psv[:, 0:1], vrow[0:1, c * P : (c + 1) * P], ident_bf[0:1, 0:1])
            nc.vector.tensor_copy(v_sb[:, c : c + 1], psv[:, 0:1])

        # ---- decoder projection -> per-(h_chunk, batch) activation bias ----
        d_nat = const_pool.tile([b_loc, hidden], F32)
        nc.gpsimd.dma_start(d_nat[:], dec[:, :])
        dT = const_pool.tile([P, kc, b_loc], F32)
        for c in range(kc):
            ps = ps_tr.tile([P, P], F32, tag="tr")
            nc.tensor.transpose(ps[:, :b_loc], d_nat[:, c * P : (c + 1) * P], ident[:b_loc, :b_loc])
            nc.vector.tensor_copy(dT[:, c, :], ps[:, :b_loc])
        bias_sb = const_pool.tile([P, kc, b_loc], F32)
        for hc in range(kc):
            pd = ps_x.tile([P, SB], F32, tag="x")
            for c in range(kc):
                nc.tensor.matmul(
                    pd[:, :b_loc],
                    w_dec_sb[:, c, hc * P : (hc + 1) * P],
                    dT[:, c, :],
                    start=(c == 0),
                    stop=(c == kc - 1),
                )
            nc.vector.tensor_scalar_add(bias_sb[:, hc, :], pd[:, :b_loc], bsum_sb[:, hc : hc + 1])

        # ---- main loop over local batches ----
        for b in range(b_loc):
            wexp = sm_pool.tile([1, seq], F32, tag="wexp")
            lpart = sm_pool.tile([1, nblk], F32, tag="lpart")
            part = sm_pool.tile([P, kc, nblk], F32, tag="part")
            for j in range(nblk):
                # E^T (h_in on partitions) comes pre-transposed from the host
                et = et_pool.tile([P, kc, sb], BF16, tag="et")
                if b == 0 and j < 2:
                    # cold start: spread the first tiles across queues
                    for c in range(kc):
                        nc.sync.dma_start(
                            et[:, c, :],
                            enc_t[b, c * P : (c + 1) * P, j * sb : (j + 1) * sb],
                        )
                else:
                    nc.sync.dma_start(
                        et[:],
                        enc_t[b, :, j * sb : (j + 1) * sb].rearrange("(c k) s -> k c s", k=P),
                    )
                # enc_proj + fused bias+tanh
                tt = tt_pool.tile([P, kc, sb], BF16, tag="tt")
                for hc in range(kc):
                    px = ps_x.tile([P, SB], F32, tag="x")
                    for c in range(kc):
                        nc.tensor.matmul(
                            px[:, :sb],
                            w_enc_sb[:, c, hc * P : (hc + 1) * P],
                            et[:, c, :],
                            start=(c == 0),
                            stop=(c == kc - 1),
                        )
                    nc.scalar.activation(
                        tt[:, hc, :], px[:, :sb], AFT.Tanh, bias=bias_sb[:, hc, b : b + 1]
                    )
                # score chunk (1, sb)
                pss = ps_sc.tile([1, SB], F32, tag="sc")
                for c in range(kc):
                    nc.tensor.matmul(
                        pss[:, :sb],
                        v_sb[:, c : c + 1],
                        tt[:, c, :],
                        start=(c == 0),
                        stop=(c == kc - 1),
                    )
                # unnormalized softmax weights for this block (scores are
                # bounded, so no max-subtraction is needed) + running sum
                nc.scalar.activation(
                    wexp[:, j * sb : (j + 1) * sb],
                    pss[:, :sb],
                    AFT.Exp,
                    accum_out=lpart[:, j : j + 1],
                )
                # online context accumulation: part[:,c,j] = sum_s w(s)*E^T(h,s)
                bc = sm_pool.tile([P, SB], F32, tag="bc")
                nc.gpsimd.partition_broadcast(bc[:, :sb], wexp[:, j * sb : (j + 1) * sb])
                for c in range(kc):
                    scratch = sm_pool.tile([P, SB], BF16, tag="scr")
                    nc.vector.scalar_tensor_tensor(
                        out=scratch[:, :sb],
                        in0=et[:, c, :],
                        scalar=1.0,
                        in1=bc[:, :sb],
                        op0=mybir.AluOpType.mult,
                        op1=mybir.AluOpType.mult,
                        accum_out=part[:, c, j : j + 1],
                    )

            # ---- per-batch tail: normalize ----
            lsum = sm_pool.tile([1, 1], F32, tag="lsum")
            nc.vector.reduce_sum(lsum[:], lpart[:], axis=mybir.AxisListType.X)
            rl = sm_pool.tile([1, 1], F32, tag="rl")
            nc.vector.reciprocal(rl[:], lsum[:])
            attn_f = sm_pool.tile([1, seq], F32, tag="attn_f")
            nc.vector.tensor_scalar_mul(attn_f[:], wexp[:], rl[:])
            nc.sync.dma_start(attn_out[b : b + 1, :], attn_f[:])

            rlb = sm_pool.tile([P, 1], F32, tag="rlb")
            nc.gpsimd.partition_broadcast(rlb[:], rl[:])
            ctxu = sm_pool.tile([P, kc], F32, tag="ctxu")
            nc.vector.reduce_sum(ctxu[:], part[:], axis=mybir.AxisListType.X)
            ctxT = sm_pool.tile([P, kc], F32, tag="ctxT")
            nc.vector.tensor_scalar_mul(ctxT[:], ctxu[:], rlb[:])
            # ctxT[p, c] = ctx[c*128+p]; one tiny PE transpose to natural order
            pst = ps_tr.tile([P, P], F32, tag="tr")
            nc.tensor.transpose(pst[:kc, :], ctxT[:], ident[:])
            ctx_sb = sm_pool.tile([kc, P], F32, tag="ctx_sb")
            nc.vector.tensor_copy(ctx_sb[:], pst[:kc, :])
            nc.sync.dma_start(ctx_out[b].rearrange("(c k) -> c k", k=P), ctx_sb[:])

    nc.compile()
    return nc


_CACHED_NC = None


def _tf32_round(x: np.ndarray) -> np.ndarray:
    """Round-to-nearest-even to the fp32r (11-bit mantissa) grid; the PE's
    fp32r mode requires pre-rounded operands (bit-matches
    neuron_dtypes.static_cast_fp32_to_fp32r)."""
    b = np.ascontiguousarray(x).view(np.uint32).astype(np.uint64)
    b = (b + 0x7FF + ((b >> 12) & 1)) & np.uint64(0xFFFFF000)
    return b.astype(np.uint32).view(np.float32)



def build_in_maps(inputs) -> list:
    import ml_dtypes

    f = lambda k: np.ascontiguousarray(np.asarray(inputs[k], dtype=np.float32))
    bf = lambda k: np.ascontiguousarray(
        np.asarray(inputs[k], dtype=np.float32).astype(ml_dtypes.bfloat16)
    )
    enc_f32 = np.asarray(inputs["encoder_outputs"], dtype=np.float32)
    enc_t = np.ascontiguousarray(np.swapaxes(enc_f32.astype(ml_dtypes.bfloat16), 1, 2))
    dec = f("decoder_hidden")
    shared = {
        "w_enc": bf("W_enc"),
        "b_enc": f("b_enc"),
        "w_dec": f("W_dec"),
        "b_dec": f("b_dec"),
        "w_v": bf("W_v"),
        "b_v": f("b_v"),
    }
    in_maps = []
    for i in range(N_CORES):
        lo, hi = i * B_LOC, (i + 1) * B_LOC
        in_maps.append(
            {
                "enc_t_in": np.ascontiguousarray(enc_t[lo:hi]),
                "dec_in": np.ascontiguousarray(dec[lo:hi]),
                **shared,
            }
        )
    return in_maps


def kernel(**inputs) -> tuple:
    global _CACHED_NC
    if _CACHED_NC is None:
        _CACHED_NC = build_kernel()
    nc = _CACHED_NC

    in_maps = build_in_maps(inputs)
    res = bass_utils.run_bass_kernel_spmd(nc, in_maps, core_ids=list(range(N_CORES)))
    ctx = np.concatenate([r["ctx_out"] for r in res.results], axis=0)
    attn = np.concatenate([r["attn_out"] for r in res.results], axis=0)
    return ctx, attn


# revision 22
# speedup vs baseline: 1.2371x; 1.0112x over previous
"""Bahdanau attention Trainium2 Bass kernel.

Problem (hardcoded): encoder_outputs (32, 4096, 512) f32, decoder_hidden
(32, 512) f32, plus tiny linear params.  Returns (context (32, 512),
attention_weights (32, 4096)).

Sharding: data-parallel over batch across 8 NeuronCores (4 batches/core),
params replicated.  The host pre-casts E to bf16 and feeds it TRANSPOSED
(b, hidden, seq) so no on-chip transposes are needed; HBM traffic per core
is ~17MB (read once).

Per-core dataflow (4 local batches x 8 seq-blocks of 512):
  - enc_proj X^T = W_enc-chunks @ E^T via bf16 matmuls accumulated in PSUM
    (h_out on partitions, seq on free), ScalarE evacuates PSUM with tanh,
    fusing the per-(h,batch) bias (dec_proj + b_enc) as the per-partition
    activation bias; output tt is bf16.
  - score = v^T @ tanh via M=1 bf16 matmuls (b_v dropped: softmax is
    shift-invariant).
  - Online softmax-free context: scores are bounded (|score| <=
    sum|W_v| ~ 23) so exp needs no max-subtraction; per block ScalarE
    computes unnormalized w = exp(score) (+ running sum via accum_out),
    GpSimd broadcasts w across partitions, and VectorE accumulates
    ctx_u[h] += sum_s w(s) E^T(h,s) via scalar_tensor_tensor accum -
    the E^T tile is consumed inside the block pipeline, so batches
    overlap with no end-of-batch stall.
  - Tail per batch: l = sum of block sums, attn = w/l (DMA'd out),
    ctx = ctx_u/l via a reciprocal broadcast + one tiny PE transpose to
    restore natural h-order.

Engine balance (measured): PE ~163us (the bottleneck: 512 enc matmuls
N=512 + 128 score matmuls), ACT ~115us, DVE ~99us, GpSimd ~50us, DMA
~55us/queue; HW exec ~197-201us on 8 cores.
"""
import sys

if "/opt/trn_rl_repo" not in sys.path:
    sys.path.insert(0, "/opt/trn_rl_repo")

from contextlib import ExitStack

import numpy as np

import concourse.bass as bass
import concourse.bacc as bacc
import concourse.tile as tile
import concourse.mybir as mybir
from concourse import bass_utils
from concourse.masks import make_identity

F32 = mybir.dt.float32
BF16 = mybir.dt.bfloat16
AFT = mybir.ActivationFunctionType

HIDDEN = 512
SEQ = 4096
BATCH = 32
N_CORES = 8
B_LOC = BATCH // N_CORES  # 4 batches per core
P = 128
KC = HIDDEN // P  # 4 hidden chunks
SB = 512  # seq block
NBLK = SEQ // SB  # 8 blocks per batch
NT = SEQ // P  # 32 seq tiles per batch


def build_kernel(b_loc=B_LOC, seq=SEQ, hidden=HIDDEN):
    """Build and compile the per-core Bass program."""
    kc = hidden // P
    nblk = seq // SB if seq >= SB else 1
    sb = min(SB, seq)
    nt = seq // P
    tpb = sb // P  # seq tiles per block

    nc = bacc.Bacc("TRN2", target_bir_lowering=False, debug=False)

    enc_t = nc.dram_tensor("enc_t_in", (b_loc, hidden, seq), BF16, kind="ExternalInput").ap()
    w_enc = nc.dram_tensor("w_enc", (hidden, hidden), BF16, kind="ExternalInput").ap()
    # host-augmented [W_dec^T | b_dec + b_enc]: bias is computed on DVE as a
    # free-dim reduction, keeping the whole setup path off the in-order PE
    w_dec_aug = nc.dram_tensor("w_dec_aug", (hidden, hidden + 1), F32, kind="ExternalInput").ap()
    dec_aug = nc.dram_tensor("dec_aug", (b_loc, hidden + 1), F32, kind="ExternalInput").ap()
    w_v = nc.dram_tensor("w_v", (hidden, 1), BF16, kind="ExternalInput").ap()

    ctx_out = nc.dram_tensor("ctx_out", (b_loc, hidden), F32, kind="ExternalOutput").ap()
    attn_out = nc.dram_tensor("attn_out", (b_loc, seq), F32, kind="ExternalOutput").ap()

    with tile.TileContext(nc) as tc, ExitStack() as ctx:
        const_pool = ctx.enter_context(tc.tile_pool(name="const", bufs=1))
        # E^T tiles: one per 512-seq block, whole batch resident for the
        # DVE context reduction; +1 so the next batch's first block streams in.
        et_pool = ctx.enter_context(tc.tile_pool(name="et", bufs=nblk + 1))
        tt_pool = ctx.enter_context(tc.tile_pool(name="tt", bufs=3))
        sm_pool = ctx.enter_context(tc.tile_pool(name="smax", bufs=2))

        ps_tr = ctx.enter_context(tc.tile_pool(name="ps_tr", bufs=1, space="PSUM"))
        ps_x = ctx.enter_context(tc.tile_pool(name="ps_x", bufs=4, space="PSUM"))
        ps_sc = ctx.enter_context(tc.tile_pool(name="ps_sc", bufs=2, space="PSUM"))

        # ---- constants / weights ----
        ident = const_pool.tile([P, P], F32)
        make_identity(nc, ident[:])

        w_enc_sb = const_pool.tile([P, kc, hidden], BF16)
        for c in range(kc):
            nc.scalar.dma_start(w_enc_sb[:, c, :], w_enc[c * P : (c + 1) * P, :])
        vrow = const_pool.tile([1, hidden], BF16)
        nc.gpsimd.dma_start(vrow[:], w_v.rearrange("h o -> o h"))
        ident_bf = const_pool.tile([P, P], BF16)
        nc.vector.tensor_copy(ident_bf[:], ident[:])
        v_sb = const_pool.tile([P, kc], BF16)
        for c in range(kc):
            psv = ps_tr.tile([P, P], BF16, tag="tr")
            nc.tensor.transpose(psv[:, 0:1], vrow[0:1, c * P : (c + 1) * P], ident_bf[0:1, 0:1])
            nc.vector.tensor_copy(v_sb[:, c : c + 1], psv[:, 0:1])

        # ---- decoder projection on DVE (no PE): for each batch row,
        # bias[h] = sum_a W_dec_aug^T[h, a] * dec_aug[b, a] ----
        wda_sb = const_pool.tile([P, kc, hidden + 1], F32)
        nc.gpsimd.dma_start(wda_sb[:], w_dec_aug.rearrange("(c k) a -> k c a", k=P))
        bias_sb = const_pool.tile([P, kc, b_loc], F32)
        for b in range(b_loc):
            da_row = const_pool.tile([1, hidden + 1], F32, name=f"da_row{b}")
            nc.gpsimd.dma_start(da_row[:], dec_aug[b : b + 1, :])
            db = sm_pool.tile([P, hidden + 1], F32, tag="db")
            nc.gpsimd.partition_broadcast(db[:], da_row[:])
            for c in range(kc):
                scrb = sm_pool.tile([P, hidden + 1], F32, tag="scrb")
                nc.vector.scalar_tensor_tensor(
                    out=scrb[:],
                    in0=wda_sb[:, c, :],
                    scalar=1.0,
                    in1=db[:],
                    op0=mybir.AluOpType.mult,
                    op1=mybir.AluOpType.mult,
                    accum_out=bias_sb[:, c, b : b + 1],
                )

        # ---- main loop over local batches ----
        for b in range(b_loc):
            wexp = sm_pool.tile([1, seq], F32, tag="wexp")
            lpart = sm_pool.tile([1, nblk], F32, tag="lpart")
            part = sm_pool.tile([P, kc, nblk], F32, tag="part")
            for j in range(nblk):
                # E^T (h_in on partitions) comes pre-transposed from the host
                et = et_pool.tile([P, kc, sb], BF16, tag="et")
                if b == 0 and j < 2:
                    # cold start: spread the first tiles across queues
                    for c in range(kc):
                        nc.sync.dma_start(
                            et[:, c, :],
                            enc_t[b, c * P : (c + 1) * P, j * sb : (j + 1) * sb],
                        )
                else:
                    nc.sync.dma_start(
                        et[:],
                        enc_t[b, :, j * sb : (j + 1) * sb].rearrange("(c k) s -> k c s", k=P),
                    )
                # enc_proj + fused bias+tanh
                tt = tt_pool.tile([P, kc, sb], BF16, tag="tt")
                for hc in range(kc):
                    px = ps_x.tile([P, SB], F32, tag="x")
                    for c in range(kc):
                        nc.tensor.matmul(
                            px[:, :sb],
                            w_enc_sb[:, c, hc * P : (hc + 1) * P],
                            et[:, c, :],
                            start=(c == 0),
                            stop=(c == kc - 1),
                        )
                    nc.scalar.activation(
                        tt[:, hc, :], px[:, :sb], AFT.Tanh, bias=bias_sb[:, hc, b : b + 1]
                    )
                # score chunk (1, sb)
                pss = ps_sc.tile([1, SB], F32, tag="sc")
                for c in range(kc):
                    nc.tensor.matmul(
                        pss[:, :sb],
                        v_sb[:, c : c + 1],
                        tt[:, c, :],
                        start=(c == 0),
                        stop=(c == kc - 1),
                    )
                # unnormalized softmax weights for this block (scores are
                # bounded, so no max-subtraction is needed) + running sum
                nc.scalar.activation(
                    wexp[:, j * sb : (j + 1) * sb],
                    pss[:, :sb],
                    AFT.Exp,
                    accum_out=lpart[:, j : j + 1],
                )
                # online context accumulation: part[:,c,j] = sum_s w(s)*E^T(h,s)
                bc = sm_pool.tile([P, SB], F32, tag="bc")
                nc.gpsimd.partition_broadcast(bc[:, :sb], wexp[:, j * sb : (j + 1) * sb])
                for c in range(kc):
                    scratch = sm_pool.tile([P, SB], BF16, tag="scr")
                    nc.vector.scalar_tensor_tensor(
                        out=scratch[:, :sb],
                        in0=et[:, c, :],
                        scalar=1.0,
                        in1=bc[:, :sb],
                        op0=mybir.AluOpType.mult,
                        op1=mybir.AluOpType.mult,
                        accum_out=part[:, c, j : j + 1],
                    )

            # ---- per-batch tail: normalize ----
            lsum = sm_pool.tile([1, 1], F32, tag="lsum")
            nc.vector.reduce_sum(lsum[:], lpart[:], axis=mybir.AxisListType.X)
            rl = sm_pool.tile([1, 1], F32, tag="rl")
            nc.vector.reciprocal(rl[:], lsum[:])
            attn_f = sm_pool.tile([1, seq], F32, tag="attn_f")
            nc.vector.tensor_scalar_mul(attn_f[:], wexp[:], rl[:])
            nc.sync.dma_start(attn_out[b : b + 1, :], attn_f[:])

            rlb = sm_pool.tile([P, 1], F32, tag="rlb")
            nc.gpsimd.partition_broadcast(rlb[:], rl[:])
            ctxu = sm_pool.tile([P, kc], F32, tag="ctxu")
            nc.vector.reduce_sum(ctxu[:], part[:], axis=mybir.AxisListType.X)
            ctxT = sm_pool.tile([P, kc], F32, tag="ctxT")
            nc.vector.tensor_scalar_mul(ctxT[:], ctxu[:], rlb[:])
            # ctxT[p, c] = ctx[c*128+p]; one tiny PE transpose to natural order
            pst = ps_tr.tile([P, P], F32, tag="tr")
            nc.tensor.transpose(pst[:kc, :], ctxT[:], ident[:])
            ctx_sb = sm_pool.tile([kc, P], F32, tag="ctx_sb")
            nc.vector.tensor_copy(ctx_sb[:], pst[:kc, :])
            nc.sync.dma_start(ctx_out[b].rearrange("(c k) -> c k", k=P), ctx_sb[:])

    nc.compile()
    return nc


_CACHED_NC = None


def _tf32_round(x: np.ndarray) -> np.ndarray:
    """Round-to-nearest-even to the fp32r (11-bit mantissa) grid; the PE's
    fp32r mode requires pre-rounded operands (bit-matches
    neuron_dtypes.static_cast_fp32_to_fp32r)."""
    b = np.ascontiguousarray(x).view(np.uint32).astype(np.uint64)
    b = (b + 0x7FF + ((b >> 12) & 1)) & np.uint64(0xFFFFF000)
    return b.astype(np.uint32).view(np.float32)



def build_in_maps(inputs) -> list:
    import ml_dtypes

    f = lambda k: np.ascontiguousarray(np.asarray(inputs[k], dtype=np.float32))
    bf = lambda k: np.ascontiguousarray(
        np.asarray(inputs[k], dtype=np.float32).astype(ml_dtypes.bfloat16)
    )
    enc_f32 = np.asarray(inputs["encoder_outputs"], dtype=np.float32)
    enc_t = np.ascontiguousarray(np.swapaxes(enc_f32.astype(ml_dtypes.bfloat16), 1, 2))
    dec = f("decoder_hidden")
    w_dec_aug = np.ascontiguousarray(
        np.concatenate(
            [f("W_dec").T, (f("b_dec") + f("b_enc"))[:, None]], axis=1
        ).astype(np.float32)
    )
    dec_aug = np.ascontiguousarray(
        np.concatenate([dec, np.ones((dec.shape[0], 1), np.float32)], axis=1)
    )
    shared = {
        "w_enc": bf("W_enc"),
        "w_dec_aug": w_dec_aug,
        "w_v": bf("W_v"),
    }
    in_maps = []
    for i in range(N_CORES):
        lo, hi = i * B_LOC, (i + 1) * B_LOC
        in_maps.append(
            {
                "enc_t_in": np.ascontiguousarray(enc_t[lo:hi]),
                "dec_aug": np.ascontiguousarray(dec_aug[lo:hi]),
                **shared,
            }
        )
    return in_maps


def kernel(**inputs) -> tuple:
    global _CACHED_NC
    if _CACHED_NC is None:
        _CACHED_NC = build_kernel()
    nc = _CACHED_NC

    in_maps = build_in_maps(inputs)
    res = bass_utils.run_bass_kernel_spmd(nc, in_maps, core_ids=list(range(N_CORES)))
    ctx = np.concatenate([r["ctx_out"] for r in res.results], axis=0)
    attn = np.concatenate([r["attn_out"] for r in res.results], axis=0)
    return ctx, attn


# revision 23
# speedup vs baseline: 1.3098x; 1.0588x over previous
"""Bahdanau attention Trainium2 Bass kernel.

Problem (hardcoded): encoder_outputs (32, 4096, 512) f32, decoder_hidden
(32, 512) f32, plus tiny linear params.  Returns (context (32, 512),
attention_weights (32, 4096)).

Sharding: data-parallel over batch across 8 NeuronCores (4 batches/core),
params replicated.  The host pre-casts E to bf16 and feeds it TRANSPOSED
(b, hidden, seq) so no on-chip transposes are needed; HBM traffic per core
is ~17MB (read once).

Per-core dataflow (4 local batches x 8 seq-blocks of 512):
  - enc_proj X^T = W_enc-chunks @ E^T via bf16 matmuls accumulated in PSUM
    (h_out on partitions, seq on free), ScalarE evacuates PSUM with tanh,
    fusing the per-(h,batch) bias (dec_proj + b_enc) as the per-partition
    activation bias; output tt is bf16.
  - score = v^T @ tanh via M=1 bf16 matmuls (b_v dropped: softmax is
    shift-invariant).
  - Online softmax-free context: scores are bounded (|score| <=
    sum|W_v| ~ 23) so exp needs no max-subtraction; per block ScalarE
    computes unnormalized w = exp(score) (+ running sum via accum_out),
    GpSimd broadcasts w across partitions, and VectorE accumulates
    ctx_u[h] += sum_s w(s) E^T(h,s) via scalar_tensor_tensor accum -
    the E^T tile is consumed inside the block pipeline, so batches
    overlap with no end-of-batch stall.
  - Tail per batch: l = sum of block sums, attn = w/l (DMA'd out),
    ctx = ctx_u/l via a reciprocal broadcast + one tiny PE transpose to
    restore natural h-order.

Engine balance (measured): PE ~163us (the bottleneck: 512 enc matmuls
N=512 + 128 score matmuls), ACT ~115us, DVE ~99us, GpSimd ~50us, DMA
~55us/queue; HW exec ~197-201us on 8 cores.
"""
import sys

if "/opt/trn_rl_repo" not in sys.path:
    sys.path.insert(0, "/opt/trn_rl_repo")

from contextlib import ExitStack

import numpy as np

import concourse.bass as bass
import concourse.bacc as bacc
import concourse.tile as tile
import concourse.mybir as mybir
from concourse import bass_utils
from concourse.masks import make_identity

F32 = mybir.dt.float32
BF16 = mybir.dt.bfloat16
AFT = mybir.ActivationFunctionType

HIDDEN = 512
SEQ = 4096
BATCH = 32
N_CORES = 8
B_LOC = BATCH // N_CORES  # 4 batches per core
P = 128
KC = HIDDEN // P  # 4 hidden chunks
SB = 512  # seq block
NBLK = SEQ // SB  # 8 blocks per batch
NT = SEQ // P  # 32 seq tiles per batch


def build_kernel(b_loc=B_LOC, seq=SEQ, hidden=HIDDEN):
    """Build and compile the per-core Bass program."""
    kc = hidden // P
    nblk = seq // SB if seq >= SB else 1
    sb = min(SB, seq)
    nt = seq // P
    tpb = sb // P  # seq tiles per block

    nc = bacc.Bacc("TRN2", target_bir_lowering=False, debug=False)

    enc_t = nc.dram_tensor("enc_t_in", (b_loc, hidden, seq), BF16, kind="ExternalInput").ap()
    w_enc = nc.dram_tensor("w_enc", (hidden, hidden), BF16, kind="ExternalInput").ap()
    # host-augmented [W_dec^T | b_dec + b_enc]: bias is computed on DVE as a
    # free-dim reduction, keeping the whole setup path off the in-order PE
    w_dec_aug = nc.dram_tensor("w_dec_aug", (hidden, hidden + 1), F32, kind="ExternalInput").ap()
    dec_aug = nc.dram_tensor("dec_aug", (b_loc, hidden + 1), F32, kind="ExternalInput").ap()
    w_v = nc.dram_tensor("w_v", (hidden, 1), BF16, kind="ExternalInput").ap()

    ctx_out = nc.dram_tensor("ctx_out", (b_loc, hidden), F32, kind="ExternalOutput").ap()
    attn_out = nc.dram_tensor("attn_out", (b_loc, seq), F32, kind="ExternalOutput").ap()

    with tile.TileContext(nc) as tc, ExitStack() as ctx:
        const_pool = ctx.enter_context(tc.tile_pool(name="const", bufs=1))
        # E^T tiles: one per 512-seq block, whole batch resident for the
        # DVE context reduction; +1 so the next batch's first block streams in.
        et_pool = ctx.enter_context(tc.tile_pool(name="et", bufs=nblk + 1))
        tt_pool = ctx.enter_context(tc.tile_pool(name="tt", bufs=3))
        sm_pool = ctx.enter_context(tc.tile_pool(name="smax", bufs=2))

        ps_tr = ctx.enter_context(tc.tile_pool(name="ps_tr", bufs=1, space="PSUM"))
        ps_x = ctx.enter_context(tc.tile_pool(name="ps_x", bufs=4, space="PSUM"))
        ps_sc = ctx.enter_context(tc.tile_pool(name="ps_sc", bufs=2, space="PSUM"))

        # ---- constants / weights ----
        ident = const_pool.tile([P, P], F32)
        make_identity(nc, ident[:])

        w_enc_sb = const_pool.tile([P, kc, hidden], BF16)
        for c in range(kc):
            nc.scalar.dma_start(w_enc_sb[:, c, :], w_enc[c * P : (c + 1) * P, :])
        vrow = const_pool.tile([1, hidden], BF16)
        nc.gpsimd.dma_start(vrow[:], w_v.rearrange("h o -> o h"))
        ident_bf = const_pool.tile([P, P], BF16)
        nc.vector.tensor_copy(ident_bf[:], ident[:])
        v_sb = const_pool.tile([P, kc], BF16)
        for c in range(kc):
            psv = ps_tr.tile([P, P], BF16, tag="tr")
            nc.tensor.transpose(psv[:, 0:1], vrow[0:1, c * P : (c + 1) * P], ident_bf[0:1, 0:1])
            nc.vector.tensor_copy(v_sb[:, c : c + 1], psv[:, 0:1])

        # ---- decoder projection on DVE (no PE): for each batch row,
        # bias[h] = sum_a W_dec_aug^T[h, a] * dec_aug[b, a] ----
        wda_sb = const_pool.tile([P, kc, hidden + 1], F32)
        for c in range(kc):
            eng = nc.sync if c % 2 == 0 else nc.scalar
            eng.dma_start(wda_sb[:, c, :], w_dec_aug[c * P : (c + 1) * P, :])
        bias_sb = const_pool.tile([P, kc, b_loc], F32)
        for b in range(b_loc):
            da_row = const_pool.tile([1, hidden + 1], F32, name=f"da_row{b}")
            nc.sync.dma_start(da_row[:], dec_aug[b : b + 1, :])
            db = sm_pool.tile([P, hidden + 1], F32, tag="db")
            nc.gpsimd.partition_broadcast(db[:], da_row[:])
            for c in range(kc):
                scrb = sm_pool.tile([P, hidden + 1], F32, tag="scrb")
                nc.vector.scalar_tensor_tensor(
                    out=scrb[:],
                    in0=wda_sb[:, c, :],
                    scalar=1.0,
                    in1=db[:],
                    op0=mybir.AluOpType.mult,
                    op1=mybir.AluOpType.mult,
                    accum_out=bias_sb[:, c, b : b + 1],
                )

        # ---- main loop over local batches ----
        for b in range(b_loc):
            wexp = sm_pool.tile([1, seq], F32, tag="wexp")
            lpart = sm_pool.tile([1, nblk], F32, tag="lpart")
            part = sm_pool.tile([P, kc, nblk], F32, tag="part")
            for j in range(nblk):
                # E^T (h_in on partitions) comes pre-transposed from the host
                et = et_pool.tile([P, kc, sb], BF16, tag="et")
                if b == 0 and j < 2:
                    # cold start: spread the first tiles across queues
                    for c in range(kc):
                        nc.sync.dma_start(
                            et[:, c, :],
                            enc_t[b, c * P : (c + 1) * P, j * sb : (j + 1) * sb],
                        )
                else:
                    nc.sync.dma_start(
                        et[:],
                        enc_t[b, :, j * sb : (j + 1) * sb].rearrange("(c k) s -> k c s", k=P),
                    )
                # enc_proj + fused bias+tanh
                tt = tt_pool.tile([P, kc, sb], BF16, tag="tt")
                for hc in range(kc):
                    px = ps_x.tile([P, SB], F32, tag="x")
                    for c in range(kc):
                        nc.tensor.matmul(
                            px[:, :sb],
                            w_enc_sb[:, c, hc * P : (hc + 1) * P],
                            et[:, c, :],
                            start=(c == 0),
                            stop=(c == kc - 1),
                        )
                    nc.scalar.activation(
                        tt[:, hc, :], px[:, :sb], AFT.Tanh, bias=bias_sb[:, hc, b : b + 1]
                    )
                # score chunk (1, sb)
                pss = ps_sc.tile([1, SB], F32, tag="sc")
                for c in range(kc):
                    nc.tensor.matmul(
                        pss[:, :sb],
                        v_sb[:, c : c + 1],
                        tt[:, c, :],
                        start=(c == 0),
                        stop=(c == kc - 1),
                    )
                # unnormalized softmax weights for this block (scores are
                # bounded, so no max-subtraction is needed) + running sum
                nc.scalar.activation(
                    wexp[:, j * sb : (j + 1) * sb],
                    pss[:, :sb],
                    AFT.Exp,
                    accum_out=lpart[:, j : j + 1],
                )
                # online context accumulation: part[:,c,j] = sum_s w(s)*E^T(h,s)
                bc = sm_pool.tile([P, SB], F32, tag="bc")
                nc.gpsimd.partition_broadcast(bc[:, :sb], wexp[:, j * sb : (j + 1) * sb])
                for c in range(kc):
                    scratch = sm_pool.tile([P, SB], BF16, tag="scr")
                    nc.vector.scalar_tensor_tensor(
                        out=scratch[:, :sb],
                        in0=et[:, c, :],
                        scalar=1.0,
                        in1=bc[:, :sb],
                        op0=mybir.AluOpType.mult,
                        op1=mybir.AluOpType.mult,
                        accum_out=part[:, c, j : j + 1],
                    )

            # ---- per-batch tail: normalize ----
            lsum = sm_pool.tile([1, 1], F32, tag="lsum")
            nc.vector.reduce_sum(lsum[:], lpart[:], axis=mybir.AxisListType.X)
            rl = sm_pool.tile([1, 1], F32, tag="rl")
            nc.vector.reciprocal(rl[:], lsum[:])
            attn_f = sm_pool.tile([1, seq], F32, tag="attn_f")
            nc.vector.tensor_scalar_mul(attn_f[:], wexp[:], rl[:])
            nc.sync.dma_start(attn_out[b : b + 1, :], attn_f[:])

            rlb = sm_pool.tile([P, 1], F32, tag="rlb")
            nc.gpsimd.partition_broadcast(rlb[:], rl[:])
            ctxu = sm_pool.tile([P, kc], F32, tag="ctxu")
            nc.vector.reduce_sum(ctxu[:], part[:], axis=mybir.AxisListType.X)
            ctxT = sm_pool.tile([P, kc], F32, tag="ctxT")
            nc.vector.tensor_scalar_mul(ctxT[:], ctxu[:], rlb[:])
            # ctxT[p, c] = ctx[c*128+p]; one tiny PE transpose to natural order
            pst = ps_tr.tile([P, P], F32, tag="tr")
            nc.tensor.transpose(pst[:kc, :], ctxT[:], ident[:])
            ctx_sb = sm_pool.tile([kc, P], F32, tag="ctx_sb")
            nc.vector.tensor_copy(ctx_sb[:], pst[:kc, :])
            nc.sync.dma_start(ctx_out[b].rearrange("(c k) -> c k", k=P), ctx_sb[:])

    nc.compile()
    return nc


_CACHED_NC = None


def _tf32_round(x: np.ndarray) -> np.ndarray:
    """Round-to-nearest-even to the fp32r (11-bit mantissa) grid; the PE's
    fp32r mode requires pre-rounded operands (bit-matches
    neuron_dtypes.static_cast_fp32_to_fp32r)."""
    b = np.ascontiguousarray(x).view(np.uint32).astype(np.uint64)
    b = (b + 0x7FF + ((b >> 12) & 1)) & np.uint64(0xFFFFF000)
    return b.astype(np.uint32).view(np.float32)



def build_in_maps(inputs) -> list:
    import ml_dtypes

    f = lambda k: np.ascontiguousarray(np.asarray(inputs[k], dtype=np.float32))
    bf = lambda k: np.ascontiguousarray(
        np.asarray(inputs[k], dtype=np.float32).astype(ml_dtypes.bfloat16)
    )
    enc_f32 = np.asarray(inputs["encoder_outputs"], dtype=np.float32)
    enc_t = np.ascontiguousarray(np.swapaxes(enc_f32.astype(ml_dtypes.bfloat16), 1, 2))
    dec = f("decoder_hidden")
    w_dec_aug = np.ascontiguousarray(
        np.concatenate(
            [f("W_dec").T, (f("b_dec") + f("b_enc"))[:, None]], axis=1
        ).astype(np.float32)
    )
    dec_aug = np.ascontiguousarray(
        np.concatenate([dec, np.ones((dec.shape[0], 1), np.float32)], axis=1)
    )
    shared = {
        "w_enc": bf("W_enc"),
        "w_dec_aug": w_dec_aug,
        "w_v": bf("W_v"),
    }
    in_maps = []
    for i in range(N_CORES):
        lo, hi = i * B_LOC, (i + 1) * B_LOC
        in_maps.append(
            {
                "enc_t_in": np.ascontiguousarray(enc_t[lo:hi]),
                "dec_aug": np.ascontiguousarray(dec_aug[lo:hi]),
                **shared,
            }
        )
    return in_maps


def kernel(**inputs) -> tuple:
    global _CACHED_NC
    if _CACHED_NC is None:
        _CACHED_NC = build_kernel()
    nc = _CACHED_NC

    in_maps = build_in_maps(inputs)
    res = bass_utils.run_bass_kernel_spmd(nc, in_maps, core_ids=list(range(N_CORES)))
    ctx = np.concatenate([r["ctx_out"] for r in res.results], axis=0)
    attn = np.concatenate([r["attn_out"] for r in res.results], axis=0)
    return ctx, attn
